# revision 13
# baseline (speedup 1.0000x reference)
"""
Trainium2 Bass kernel for DynamicGraphAttention
(softmax(Hn Wq^T (Hn Wk^T)^T / sqrt(D) + eta*logit(clip(A)) masked)).

Shapes (hardcoded):
  Hn     [16, 2048, 256] f32
  A_stat [2048, 2048]    f32
  M_mask [2048, 2048]    int32
  Wq, Wk [256, 256]      f32
  out    [16, 2048, 2048] f32

Sharding across 8 NeuronCores: 4 batch-groups x 2 seq(query)-groups.
Core c handles batches of group bg = c // 2 (4 each) and query rows
[qg*1024:(qg+1)*1024] (qg = c % 2).

Key ideas vs the v1 kernel (161 us):
  * Output leaves the device in bf16 (16 MB/core instead of 32 MB f32);
    host converts to f32. bf16 keeps full exponent range so tiny softmax
    probabilities survive; adds <= 0.4% relative rounding.
  * The bias eta*logit(clip(a)) + (-inf) masking is folded
    multiplicatively: softmax(S+B) = expB*exp(S)/sum with
    expB = (a/(1-a))*mask precomputed on host in fp16 (4 MB/core).
    Kills the on-device log prep AND the fp32r identity-matmul bias
    add, and keeps every DVE pass in 16-bit (2x rate) SBUF operands —
    a DVE pass reading PSUM f32 measured 2.3 us/tile vs 0.73 for fp16.
  * The query-side slice of Hn^T is taken from the already-resident hnt
    tiles instead of being shipped twice. Per-core query windows differ,
    so the host rotates the key axis per core (softmax rows are
    permutation-invariant; host un-rotates the output columns).
  * ACT does only Exp (single activation table set, loaded once during
    the input DMAs by a tiny warmup activation).

Per-core device loop (32 output tiles of [128 q, 2048 k]):
  G  = (Wq^T Wk)/16          fp32 matmul -> fp16   [256,256]   (PE)
  VT = G^T Hq^T  per batch   fp16                  [256,1024]  (PE)
  S  = VT.T @ HnT            fp16 MMs, PSUM f32    [128,2048]  (PE)
  p  = exp(S)                psum -> sbuf fp16                 (ACT)
  t, rs = p*expB, rowsum     scalar_tensor_tensor w/ accum     (DVE)
  o  = t * (1/rs)            -> bf16                           (DVE)
  DMA out 1 MB per q-tile pair                                 (SWDGE)
"""

import math

import numpy as np

import concourse.bass as bass
import concourse.bacc as bacc
import concourse.tile as tile
from concourse import mybir
from concourse import bass_utils

F32 = mybir.dt.float32
BF16 = mybir.dt.bfloat16
FP16 = mybir.dt.float16

B_FULL = 16
N = 2048
D = 256
NBG = 4   # batch groups
NQG = 2   # seq (query-row) groups
NB = B_FULL // NBG        # batches per core = 4
NQ = N // NQG             # query rows per core = 1024
NQT = NQ // 128           # q tiles per core = 8
HCH = 512                 # moving-operand chunk for the S matmuls
EPS = 1e-3
SCALE = 1.0 / math.sqrt(float(D))  # 1/16

_CACHE = {}


def _build():
    nc = bacc.Bacc("TRN2", debug=False, enable_asserts=False)

    hnt_d = nc.dram_tensor("hnt", [NB, D, N], FP16, kind="ExternalInput").ap()
    eb_d = nc.dram_tensor("eb", [NQ, N], FP16, kind="ExternalInput").ap()
    wq_d = nc.dram_tensor("wq", [D, D], F32, kind="ExternalInput").ap()
    wk_d = nc.dram_tensor("wk", [D, D], F32, kind="ExternalInput").ap()
    o_d = nc.dram_tensor("o", [NB, NQ, N], BF16, kind="ExternalOutput").ap()

    MUL = mybir.AluOpType.mult
    ADD = mybir.AluOpType.add

    with tile.TileContext(nc) as tc:
        with (
            tc.tile_pool(name="consts", bufs=1) as consts,
            tc.tile_pool(name="hntp", bufs=4) as hntp,
            tc.tile_pool(name="ebp", bufs=1) as ebp,
            tc.tile_pool(name="vtp", bufs=4) as vtp,
            tc.tile_pool(name="pp", bufs=3) as pp,
            tc.tile_pool(name="tp", bufs=3) as tp,
            tc.tile_pool(name="op", bufs=2) as op_pool,
            tc.tile_pool(name="rsp", bufs=4) as rsp,
            tc.tile_pool(name="ps_s", bufs=2, space="PSUM") as ps_s,
        ):
            # ACT warmup: force the Exp table set to load while DMAs run
            dwi = consts.tile([128, 1], F32, tag="dwi")
            nc.vector.memset(dwi, 0.0)
            dwo = consts.tile([128, 1], F32, tag="dwo")
            nc.scalar.activation(
                out=dwo, in_=dwi, func=mybir.ActivationFunctionType.Exp
            )

            # ---- input DMAs (HWDGE, issue order == drain order) ----
            wq_sb = consts.tile([128, 2, D], F32, tag="wq")
            nc.sync.dma_start(out=wq_sb, in_=wq_d.rearrange("(c p) d -> p c d", p=128))
            wk_sb = consts.tile([128, 2, D], F32, tag="wk")
            nc.sync.dma_start(out=wk_sb, in_=wk_d.rearrange("(c p) d -> p c d", p=128))

            hnt = [
                hntp.tile([128, 2, N], FP16, tag="hnt", name=f"hnt{b}")
                for b in range(NB)
            ]
            eb = ebp.tile([128, NQT, N], FP16, tag="eb")
            eb_r = eb_d.rearrange("(t p) k -> p t k", p=128)

            def load_hnt(b):
                nc.sync.dma_start(
                    out=hnt[b], in_=hnt_d[b].rearrange("(c p) n -> p c n", p=128)
                )

            load_hnt(0)
            nc.sync.dma_start(out=eb[:, 0:4, :], in_=eb_r[:, 0:4, :])
            load_hnt(1)
            nc.sync.dma_start(out=eb[:, 4:8, :], in_=eb_r[:, 4:8, :])
            load_hnt(2)
            load_hnt(3)

            # ---- G = (Wq^T Wk) * SCALE : 2 tiles [128(i), 256(j)] fp16 ----
            g = []
            for i in range(2):
                gp = ps_s.tile([128, N], F32, tag="s", name=f"gp{i}")
                for e in range(2):
                    nc.tensor.matmul(
                        gp[:, :D],
                        lhsT=wq_sb[:, e, i * 128:(i + 1) * 128],
                        rhs=wk_sb[:, e, :],
                        start=(e == 0),
                        stop=(e == 1),
                    )
                g_i = consts.tile([128, D], FP16, tag=f"g{i}", name=f"g{i}")
                nc.vector.tensor_scalar(
                    out=g_i, in0=gp[:, :D], scalar1=SCALE, scalar2=None, op0=MUL
                )
                g.append(g_i)

            # ---- VT[b] = (Hq G) laid out [d, q] : 2 tiles [128, 1024] ----
            def emit_vt(b):
                vt = []
                for j in range(2):
                    vp = ps_s.tile([128, N], F32, tag="s", name=f"vp{b}{j}")
                    for c in range(NQ // HCH):
                        csl = slice(c * HCH, (c + 1) * HCH)
                        for i in range(2):
                            nc.tensor.matmul(
                                vp[:, :NQ][:, csl],
                                lhsT=g[i][:, j * 128:(j + 1) * 128],
                                rhs=hnt[b][:, i, :NQ][:, csl],
                                start=(i == 0),
                                stop=(i == 1),
                            )
                    vt_j = vtp.tile([128, NQ], FP16, tag="vt", name=f"vt{b}_{j}")
                    nc.vector.tensor_copy(out=vt_j, in_=vp[:, :NQ])
                    vt.append(vt_j)
                return vt

            ot = [None]

            def emit_qtile(b, qt, vt):
                qsl = slice(qt * 128, (qt + 1) * 128)
                s_ps = ps_s.tile([128, N], F32, tag="s", name=f"s{b}{qt}")
                for j in range(2):
                    for c in range(N // HCH):
                        csl = slice(c * HCH, (c + 1) * HCH)
                        nc.tensor.matmul(
                            s_ps[:, csl],
                            lhsT=vt[j][:, qsl],
                            rhs=hnt[b][:, j, csl],
                            start=(j == 0),
                            stop=(j == 1),
                        )
                p = pp.tile([128, N], FP16, tag="p", name=f"p{b}{qt}")
                nc.scalar.activation(
                    out=p, in_=s_ps, func=mybir.ActivationFunctionType.Exp
                )
                t = tp.tile([128, N], FP16, tag="t", name=f"t{b}{qt}")
                rs = rsp.tile([128, 1], F32, tag="rs", name=f"rs{b}{qt}")
                nc.vector.scalar_tensor_tensor(
                    out=t,
                    in0=p,
                    scalar=1.0,
                    in1=eb[:, qt, :],
                    op0=MUL,
                    op1=MUL,
                    accum_out=rs,
                )
                rinv = rsp.tile([128, 1], F32, tag="ri", name=f"ri{b}{qt}")
                nc.vector.reciprocal(out=rinv, in_=rs)
                if qt % 2 == 0:
                    ot[0] = op_pool.tile(
                        [128, 2, N], BF16, tag="o", name=f"o{b}{qt // 2}"
                    )
                # normalize on GPSIMD (Pool) — DVE is the busiest engine;
                # Pool runs tensor_scalar at ~1 cyc/elem and is otherwise
                # only issuing the output-DMA descriptors
                nc.gpsimd.tensor_scalar(
                    out=ot[0][:, qt % 2, :], in0=t, scalar1=rinv, scalar2=None, op0=MUL
                )
                if qt % 2 == 1:
                    qp = qt // 2
                    nc.gpsimd.dma_start(
                        out=o_d[b, qp * 256:(qp + 1) * 256, :].rearrange(
                            "(t p) k -> p t k", p=128
                        ),
                        in_=ot[0],
                    )

            vt_cur = emit_vt(0)
            for b in range(NB):
                for qt in range(NQT):
                    emit_qtile(b, qt, vt_cur)
                if b + 1 < NB:
                    vt_cur = emit_vt(b + 1)
    nc.compile()
    return nc


def _get_nc():
    if "nc" not in _CACHE:
        _CACHE["nc"] = _build()
    return _CACHE["nc"]


def make_in_maps(Hn, A_stat, M_mask, Wq, Wk):
    Hn = np.asarray(Hn, dtype=np.float32)
    A = np.asarray(A_stat, dtype=np.float32)
    M = np.asarray(M_mask)
    Wq = np.ascontiguousarray(np.asarray(Wq, dtype=np.float32))
    Wk = np.ascontiguousarray(np.asarray(Wk, dtype=np.float32))
    assert Hn.shape == (B_FULL, N, D)

    a = np.clip(A, EPS, 1.0 - EPS)
    bias16 = ((a / (1.0 - a)) * (M != 0)).astype(np.float16)

    # [16, 256, 2048] transposed-node layout, fp16
    hnt_full = np.ascontiguousarray(Hn.astype(np.float16).transpose(0, 2, 1))

    in_maps = []
    for c in range(8):
        bg, qg = divmod(c, NQG)
        bsl = slice(bg * NB, (bg + 1) * NB)
        q0, q1 = qg * NQ, (qg + 1) * NQ
        h = hnt_full[bsl]
        e = bias16[q0:q1]
        if q0 == 0:
            hnt_c = h
            eb_c = np.ascontiguousarray(e)
        else:
            # rotate key axis so this core's query block sits at column 0
            hnt_c = np.ascontiguousarray(
                np.concatenate([h[:, :, q0:q1], h[:, :, :q0], h[:, :, q1:]], axis=2)
            )
            eb_c = np.ascontiguousarray(
                np.concatenate([e[:, q0:q1], e[:, :q0], e[:, q1:]], axis=1)
            )
        in_maps.append({
            "hnt": hnt_c,
            "eb": eb_c,
            "wq": Wq,
            "wk": Wk,
        })
    return in_maps


def assemble(results):
    out = np.empty((B_FULL, N, N), dtype=np.float32)
    for c in range(8):
        bg, qg = divmod(c, NQG)
        o = np.asarray(results[c]["o"]).astype(np.float32)  # [NB, NQ, N]
        bsl = slice(bg * NB, (bg + 1) * NB)
        q0, q1 = qg * NQ, (qg + 1) * NQ
        qsl = slice(q0, q1)
        if q0 == 0:
            out[bsl, qsl, :] = o
        else:
            out[bsl, qsl, q0:q1] = o[:, :, :NQ]
            out[bsl, qsl, :q0] = o[:, :, NQ:NQ + q0]
            out[bsl, qsl, q1:] = o[:, :, NQ + q0:]
    return out


def kernel(Hn, A_stat, M_mask, Wq, Wk):
    in_maps = make_in_maps(Hn, A_stat, M_mask, Wq, Wk)
    nc = _get_nc()
    res = bass_utils.run_bass_kernel_spmd(nc, in_maps, core_ids=list(range(8)))
    return assemble(res.results)


if __name__ == "__main__":
    rng = np.random.default_rng(0)
    inputs = {
        "Hn": rng.standard_normal((B_FULL, N, D), dtype=np.float32),
        "A_stat": rng.random((N, N), dtype=np.float32),
        "M_mask": rng.integers(0, 2, size=(N, N), dtype=np.int32),
        "Wq": rng.standard_normal((D, D), dtype=np.float32) / 16,
        "Wk": rng.standard_normal((D, D), dtype=np.float32) / 16,
    }
    out = kernel(**inputs)
    print(out.shape, out.dtype, out.sum())


# revision 15
# speedup vs baseline: 8.1624x; 8.1624x over previous
"""
Trainium2 Bass kernel for DynamicGraphAttention
(softmax(Hn Wq^T (Hn Wk^T)^T / sqrt(D) + eta*logit(clip(A)) masked)).

Shapes (hardcoded):
  Hn     [16, 2048, 256] f32
  A_stat [2048, 2048]    f32
  M_mask [2048, 2048]    int32
  Wq, Wk [256, 256]      f32
  out    [16, 2048, 2048] f32

Sharding across 8 NeuronCores: 4 batch-groups x 2 seq(query)-groups.
Core c handles batches of group bg = c // 2 (4 each) and query rows
[qg*1024:(qg+1)*1024] (qg = c % 2).

Key ideas vs the v1 kernel (161 us):
  * Output leaves the device in bf16 (16 MB/core instead of 32 MB f32);
    host converts to f32. bf16 keeps full exponent range so tiny softmax
    probabilities survive; adds <= 0.4% relative rounding.
  * The bias eta*logit(clip(a)) + (-inf) masking is folded
    multiplicatively: softmax(S+B) = expB*exp(S)/sum with
    expB = (a/(1-a))*mask precomputed on host in fp16 (4 MB/core).
    Kills the on-device log prep AND the fp32r identity-matmul bias
    add, and keeps every DVE pass in 16-bit (2x rate) SBUF operands —
    a DVE pass reading PSUM f32 measured 2.3 us/tile vs 0.73 for fp16.
  * The query-side slice of Hn^T is taken from the already-resident hnt
    tiles instead of being shipped twice. Per-core query windows differ,
    so the host rotates the key axis per core (softmax rows are
    permutation-invariant; host un-rotates the output columns).
  * ACT does only Exp (single activation table set, loaded once during
    the input DMAs by a tiny warmup activation).

Per-core device loop (32 output tiles of [128 q, 2048 k]):
  G  = (Wq^T Wk)/16          fp32 matmul -> fp16   [256,256]   (PE)
  VT = G^T Hq^T  per batch   fp16                  [256,1024]  (PE)
  S  = VT.T @ HnT            fp16 MMs, PSUM f32    [128,2048]  (PE)
  p  = exp(S)                psum -> sbuf fp16                 (ACT)
  t, rs = p*expB, rowsum     scalar_tensor_tensor w/ accum     (DVE)
  o  = t * (1/rs)            -> bf16                           (DVE)
  DMA out 1 MB per q-tile pair                                 (SWDGE)
"""

import math

import numpy as np

import concourse.bass as bass
import concourse.bacc as bacc
import concourse.tile as tile
from concourse import mybir
from concourse import bass_utils

F32 = mybir.dt.float32
BF16 = mybir.dt.bfloat16
FP16 = mybir.dt.float16

B_FULL = 16
N = 2048
D = 256
NBG = 4   # batch groups
NQG = 2   # seq (query-row) groups
NB = B_FULL // NBG        # batches per core = 4
NQ = N // NQG             # query rows per core = 1024
NQT = NQ // 128           # q tiles per core = 8
HCH = 512                 # moving-operand chunk for the S matmuls
EPS = 1e-3
SCALE = 1.0 / math.sqrt(float(D))  # 1/16

_CACHE = {}


def _build():
    nc = bacc.Bacc("TRN2", debug=False, enable_asserts=False)

    hnt_d = nc.dram_tensor("hnt", [NB, D, N], FP16, kind="ExternalInput").ap()
    eb_d = nc.dram_tensor("eb", [NQ, N], FP16, kind="ExternalInput").ap()
    wq_d = nc.dram_tensor("wq", [D, D], F32, kind="ExternalInput").ap()
    wk_d = nc.dram_tensor("wk", [D, D], F32, kind="ExternalInput").ap()
    o_d = nc.dram_tensor("o", [NB, NQ, N], BF16, kind="ExternalOutput").ap()

    MUL = mybir.AluOpType.mult
    ADD = mybir.AluOpType.add

    with tile.TileContext(nc) as tc:
        with (
            tc.tile_pool(name="consts", bufs=1) as consts,
            tc.tile_pool(name="hntp", bufs=4) as hntp,
            tc.tile_pool(name="ebp", bufs=1) as ebp,
            tc.tile_pool(name="vtp", bufs=4) as vtp,
            tc.tile_pool(name="pp", bufs=3) as pp,
            tc.tile_pool(name="tp", bufs=3) as tp,
            tc.tile_pool(name="op", bufs=2) as op_pool,
            tc.tile_pool(name="rsp", bufs=4) as rsp,
            tc.tile_pool(name="ps_s", bufs=2, space="PSUM") as ps_s,
        ):
            # ACT warmup: force the Exp table set to load while DMAs run
            dwi = consts.tile([128, 1], F32, tag="dwi")
            nc.vector.memset(dwi, 0.0)
            dwo = consts.tile([128, 1], F32, tag="dwo")
            nc.scalar.activation(
                out=dwo, in_=dwi, func=mybir.ActivationFunctionType.Exp
            )

            # ---- input DMAs (HWDGE, issue order == drain order) ----
            wq_sb = consts.tile([128, 2, D], F32, tag="wq")
            nc.sync.dma_start(out=wq_sb, in_=wq_d.rearrange("(c p) d -> p c d", p=128))
            wk_sb = consts.tile([128, 2, D], F32, tag="wk")
            nc.sync.dma_start(out=wk_sb, in_=wk_d.rearrange("(c p) d -> p c d", p=128))

            hnt = [
                hntp.tile([128, 2, N], FP16, tag="hnt", name=f"hnt{b}")
                for b in range(NB)
            ]
            eb = ebp.tile([128, NQT, N], FP16, tag="eb")
            eb_r = eb_d.rearrange("(t p) k -> p t k", p=128)

            def load_hnt(b):
                nc.sync.dma_start(
                    out=hnt[b], in_=hnt_d[b].rearrange("(c p) n -> p c n", p=128)
                )

            load_hnt(0)
            nc.sync.dma_start(out=eb[:, 0:4, :], in_=eb_r[:, 0:4, :])
            load_hnt(1)
            nc.sync.dma_start(out=eb[:, 4:8, :], in_=eb_r[:, 4:8, :])
            load_hnt(2)
            load_hnt(3)

            # ---- G = (Wq^T Wk) * SCALE : 2 tiles [128(i), 256(j)] fp16 ----
            g = []
            for i in range(2):
                gp = ps_s.tile([128, N], F32, tag="s", name=f"gp{i}")
                for e in range(2):
                    nc.tensor.matmul(
                        gp[:, :D],
                        lhsT=wq_sb[:, e, i * 128:(i + 1) * 128],
                        rhs=wk_sb[:, e, :],
                        start=(e == 0),
                        stop=(e == 1),
                    )
                g_i = consts.tile([128, D], FP16, tag=f"g{i}", name=f"g{i}")
                nc.vector.tensor_scalar(
                    out=g_i, in0=gp[:, :D], scalar1=SCALE, scalar2=None, op0=MUL
                )
                g.append(g_i)

            # ---- VT[b] = (Hq G) laid out [d, q] : 2 tiles [128, 1024] ----
            def emit_vt(b):
                vt = []
                for j in range(2):
                    vp = ps_s.tile([128, N], F32, tag="s", name=f"vp{b}{j}")
                    for c in range(NQ // HCH):
                        csl = slice(c * HCH, (c + 1) * HCH)
                        for i in range(2):
                            nc.tensor.matmul(
                                vp[:, :NQ][:, csl],
                                lhsT=g[i][:, j * 128:(j + 1) * 128],
                                rhs=hnt[b][:, i, :NQ][:, csl],
                                start=(i == 0),
                                stop=(i == 1),
                            )
                    vt_j = vtp.tile([128, NQ], FP16, tag="vt", name=f"vt{b}_{j}")
                    # psum->sbuf cast on ACT (Copy lives in the same
                    # activation-table set as Exp, so no table reloads);
                    # keeps the cast off the DVE, which paces the kernel
                    nc.scalar.copy(out=vt_j, in_=vp[:, :NQ])
                    vt.append(vt_j)
                return vt

            ot = [None]

            def emit_qtile(b, qt, vt):
                qsl = slice(qt * 128, (qt + 1) * 128)
                s_ps = ps_s.tile([128, N], F32, tag="s", name=f"s{b}{qt}")
                for j in range(2):
                    for c in range(N // HCH):
                        csl = slice(c * HCH, (c + 1) * HCH)
                        nc.tensor.matmul(
                            s_ps[:, csl],
                            lhsT=vt[j][:, qsl],
                            rhs=hnt[b][:, j, csl],
                            start=(j == 0),
                            stop=(j == 1),
                        )
                p = pp.tile([128, N], FP16, tag="p", name=f"p{b}{qt}")
                nc.scalar.activation(
                    out=p, in_=s_ps, func=mybir.ActivationFunctionType.Exp
                )
                t = tp.tile([128, N], FP16, tag="t", name=f"t{b}{qt}")
                rs = rsp.tile([128, 1], F32, tag="rs", name=f"rs{b}{qt}")
                nc.vector.scalar_tensor_tensor(
                    out=t,
                    in0=p,
                    scalar=1.0,
                    in1=eb[:, qt, :],
                    op0=MUL,
                    op1=MUL,
                    accum_out=rs,
                )
                rinv = rsp.tile([128, 1], F32, tag="ri", name=f"ri{b}{qt}")
                nc.vector.reciprocal(out=rinv, in_=rs)
                if qt % 2 == 0:
                    ot[0] = op_pool.tile(
                        [128, 2, N], BF16, tag="o", name=f"o{b}{qt // 2}"
                    )
                nc.vector.tensor_scalar(
                    out=ot[0][:, qt % 2, :], in0=t, scalar1=rinv, scalar2=None, op0=MUL
                )
                if qt % 2 == 1:
                    qp = qt // 2
                    nc.gpsimd.dma_start(
                        out=o_d[b, qp * 256:(qp + 1) * 256, :].rearrange(
                            "(t p) k -> p t k", p=128
                        ),
                        in_=ot[0],
                    )

            vt_cur = emit_vt(0)
            for b in range(NB):
                for qt in range(NQT):
                    emit_qtile(b, qt, vt_cur)
                if b + 1 < NB:
                    vt_cur = emit_vt(b + 1)
    nc.compile()
    return nc


def _get_nc():
    if "nc" not in _CACHE:
        _CACHE["nc"] = _build()
    return _CACHE["nc"]


def make_in_maps(Hn, A_stat, M_mask, Wq, Wk):
    Hn = np.asarray(Hn, dtype=np.float32)
    A = np.asarray(A_stat, dtype=np.float32)
    M = np.asarray(M_mask)
    Wq = np.ascontiguousarray(np.asarray(Wq, dtype=np.float32))
    Wk = np.ascontiguousarray(np.asarray(Wk, dtype=np.float32))
    assert Hn.shape == (B_FULL, N, D)

    a = np.clip(A, EPS, 1.0 - EPS)
    bias16 = ((a / (1.0 - a)) * (M != 0)).astype(np.float16)

    # [16, 256, 2048] transposed-node layout, fp16
    hnt_full = np.ascontiguousarray(Hn.astype(np.float16).transpose(0, 2, 1))

    in_maps = []
    for c in range(8):
        bg, qg = divmod(c, NQG)
        bsl = slice(bg * NB, (bg + 1) * NB)
        q0, q1 = qg * NQ, (qg + 1) * NQ
        h = hnt_full[bsl]
        e = bias16[q0:q1]
        if q0 == 0:
            hnt_c = h
            eb_c = np.ascontiguousarray(e)
        else:
            # rotate key axis so this core's query block sits at column 0
            hnt_c = np.ascontiguousarray(
                np.concatenate([h[:, :, q0:q1], h[:, :, :q0], h[:, :, q1:]], axis=2)
            )
            eb_c = np.ascontiguousarray(
                np.concatenate([e[:, q0:q1], e[:, :q0], e[:, q1:]], axis=1)
            )
        in_maps.append({
            "hnt": hnt_c,
            "eb": eb_c,
            "wq": Wq,
            "wk": Wk,
        })
    return in_maps


def assemble(results):
    out = np.empty((B_FULL, N, N), dtype=np.float32)
    for c in range(8):
        bg, qg = divmod(c, NQG)
        o = np.asarray(results[c]["o"]).astype(np.float32)  # [NB, NQ, N]
        bsl = slice(bg * NB, (bg + 1) * NB)
        q0, q1 = qg * NQ, (qg + 1) * NQ
        qsl = slice(q0, q1)
        if q0 == 0:
            out[bsl, qsl, :] = o
        else:
            out[bsl, qsl, q0:q1] = o[:, :, :NQ]
            out[bsl, qsl, :q0] = o[:, :, NQ:NQ + q0]
            out[bsl, qsl, q1:] = o[:, :, NQ + q0:]
    return out


def kernel(Hn, A_stat, M_mask, Wq, Wk):
    in_maps = make_in_maps(Hn, A_stat, M_mask, Wq, Wk)
    nc = _get_nc()
    res = bass_utils.run_bass_kernel_spmd(nc, in_maps, core_ids=list(range(8)))
    return assemble(res.results)


if __name__ == "__main__":
    rng = np.random.default_rng(0)
    inputs = {
        "Hn": rng.standard_normal((B_FULL, N, D), dtype=np.float32),
        "A_stat": rng.random((N, N), dtype=np.float32),
        "M_mask": rng.integers(0, 2, size=(N, N), dtype=np.int32),
        "Wq": rng.standard_normal((D, D), dtype=np.float32) / 16,
        "Wk": rng.standard_normal((D, D), dtype=np.float32) / 16,
    }
    out = kernel(**inputs)
    print(out.shape, out.dtype, out.sum())
